# revision 17
# baseline (speedup 1.0000x reference)
"""FAST-9 corner score (nn_FASTScore) on 8 Trainium2 NeuronCores.

Matches the jax reference bit-exactly, including the neuron-lowered int32
division rounding quirk in _detect_9_consecutive:
  (x // 2^k) gains +1 iff x >= 2^23 and (k == 0 and x odd, or k >= 2 and
  x mod 2^k == 2^k - 1); k == 1 is exact.
Folding that into the 9-consecutive-bit test over the 24-bit circular
buffer W gives, per window k (window 0 can never pass):
  k == 1 : pass iff bits 1..9 all set            (exact path)
  k >= 2 : quirk_k = b23 & (low k bits of W set)
           pass iff (win==511 & !quirk_k) | (win==510 & quirk_k)
computed bitwise as:
  E9  = 9-run mask;  QMraw = (W+1)^W = 2*lowest_zero_bit - 1
  QM  = QMraw & (all-ones if b23)          # bits k with quirk_k
  ep  = E9 & (((QM|1) & ~2) ^ 0xFFFFFF)    # bit0 killed, bit1 exempt
  qp  = ((r8>>1) & ~W) & (QM & 0xFFFC)     # win==510 & quirk, k in 2..15
  detected iff (ep | qp) != 0
Verified against the reference on all 8.3M pixels (0 mismatches) and the
division rule on 683k boundary samples x 16 divisors.

Device schedule per core (540 output rows of one image):
  host-padded [546, 1926] f32 block; 5 row tiles x 4 x-chunks of [108, 480].
  One folded DMA per chunk loads all 7 vertical shifts (overlapping-read AP).
  GPSIMD: 16 f32 diff planes (single rounding, matches oracle) + 9 of 16
  paired compares. DVE: 7 paired compares, bf16 add-tree pack (weights
  pre-applied in the 2-scalar compare), both polarities stacked [TR,2,WC]
  through a fused scalar_tensor_tensor bitwise detect chain.
"""

import sys

for _p in ("/opt/trn_rl_repo", "/root/.axon_site/_ro/trn_rl_repo"):
    if _p not in sys.path:
        sys.path.append(_p)

import numpy as np

import concourse.bass as bass
import concourse.bacc as bacc
import concourse.tile as tile
from concourse import mybir
from concourse.alu_op_type import AluOpType as Alu
from concourse.bass_utils import run_bass_kernel_spmd

THRESHOLD = 20.0
CIRCLE_OFFSETS = [(0, -3), (1, -3), (2, -2), (3, -1), (3, 0), (3, 1), (2, 2), (1, 3),
                  (0, 3), (-1, 3), (-2, 2), (-3, 1), (-3, 0), (-3, -1), (-2, -2), (-1, -3)]

N, H, W = 4, 1080, 1920
NCORES = 8
ROWS_PER_CORE = (N * H) // NCORES          # 540
PAD = 3
TR = 108                                   # output rows per row-tile
NT = ROWS_PER_CORE // TR                   # 5 row tiles
WC = 480                                   # output cols per x-chunk
NXC = W // WC                              # 4 x chunks

F32 = mybir.dt.float32
BF16 = mybir.dt.bfloat16
I32 = mybir.dt.int32

M24 = 0xFFFFFF


def _detect_quirky_stacked(nc, pool, wi2):
    """Fused quirky detect on stacked words wi2 int32 [TR, 2, WC]."""
    def it(t):
        return pool.tile([TR, 2, WC], I32, tag=t, name=t, bufs=1)

    v = nc.vector

    def stt(out, in0, s, op0, in1, op1):
        # like v.scalar_tensor_tensor but with an int32 immediate (the walrus
        # verifier requires bitvec stt imms to be integer-typed and match
        # src/dst dtype)
        v.add_instruction(mybir.InstTensorScalarPtr(
            name=nc.get_next_instruction_name(),
            is_scalar_tensor_tensor=True,
            op0=op0, op1=op1,
            ins=[v.lower_ap(in0[:]),
                 mybir.ImmediateValue(dtype=mybir.dt.int32, value=s),
                 v.lower_ap(in1[:])],
            outs=[v.lower_ap(out[:])]))

    i0, i1, i2, i3, i4, i5 = (it(f"dq{k}") for k in range(6))

    stt(i0, wi2, 1, Alu.logical_shift_right, wi2, Alu.bitwise_and)   # a: runs>=2
    stt(i1, i0, 2, Alu.logical_shift_right, i0, Alu.bitwise_and)     # b4
    stt(i0, i1, 4, Alu.logical_shift_right, i1, Alu.bitwise_and)     # r8
    stt(i2, wi2, 8, Alu.logical_shift_right, i0, Alu.bitwise_and)    # E9
    v.tensor_scalar(out=i5[:], in0=wi2[:], scalar1=1, scalar2=None,
                    op0=Alu.add)                                     # W+1
    v.tensor_tensor(out=i3[:], in0=i5[:], in1=wi2[:],
                    op=Alu.bitwise_xor)                              # QMraw=(W+1)^W
    v.tensor_scalar(out=i4[:], in0=wi2[:], scalar1=8, scalar2=31,
                    op0=Alu.logical_shift_left, op1=Alu.arith_shift_right)  # neg7
    v.tensor_tensor(out=i1[:], in0=i3[:], in1=i4[:], op=Alu.bitwise_and)    # QM
    v.tensor_scalar(out=i3[:], in0=i1[:], scalar1=1, scalar2=-3,
                    op0=Alu.bitwise_or, op1=Alu.bitwise_and)         # t1
    stt(i4, i3, M24, Alu.bitwise_xor, i2, Alu.bitwise_and)           # ep
    v.tensor_scalar(out=i2[:], in0=wi2[:], scalar1=M24, scalar2=None,
                    op0=Alu.bitwise_xor)                             # notW
    stt(i5, i0, 1, Alu.logical_shift_right, i2, Alu.bitwise_and)     # win510
    stt(i0, i1, 0xFFFC, Alu.bitwise_and, i5, Alu.bitwise_and)        # qp
    v.tensor_tensor(out=i1[:], in0=i4[:], in1=i0[:], op=Alu.bitwise_or)  # pass
    return i1


def _emit_chunk(nc, pool, pool2, sh, out_ap, r0, x0):
    """Emit ops for one [TR, WC] output chunk. sh[dy] = shifted img tile APs."""
    center = sh[0][:, PAD:PAD + WC]

    # 16 diff planes, single f32 rounding (must match oracle's circle-center).
    # GPSIMD carries these (engine otherwise idle).
    d = pool.tile([TR, 16, WC], F32, tag="d")
    for j, (dy, dx) in enumerate(CIRCLE_OFFSETS):
        nb = sh[dy][:, PAD + dx:PAD + dx + WC]
        nc.gpsimd.tensor_tensor(out=d[:, j, :], in0=nb, in1=center,
                                op=Alu.subtract)

    # Weighted bit planes, paired (j, j+8) share weight 2^j -> one 2-plane op.
    # Stacked layout: planes 0..15 dark, 16..31 bright.
    bits = pool.tile([TR, 32, WC], BF16, tag="bits")
    for j in range(8):
        wj = float(1 << j)
        eng = nc.vector if j < 2 else nc.gpsimd
        eng.tensor_scalar(out=bits[:, j:16:8, :], in0=d[:, j::8, :],
                          scalar1=THRESHOLD, scalar2=wj,
                          op0=Alu.is_ge, op1=Alu.mult)
        nc.gpsimd.tensor_scalar(out=bits[:, 16 + j:32:8, :], in0=d[:, j::8, :],
                                scalar1=-THRESHOLD, scalar2=wj,
                                op0=Alu.is_le, op1=Alu.mult)

    # bf16 add tree (weights pre-applied; all partial sums <= 255 exact)
    t1 = pool.tile([TR, 16, WC], BF16, tag="t1")
    nc.vector.tensor_tensor(out=t1[:], in0=bits[:, 0:32:2, :],
                            in1=bits[:, 1:32:2, :], op=Alu.add)
    t2 = pool.tile([TR, 8, WC], BF16, tag="t2")
    nc.vector.tensor_tensor(out=t2[:], in0=t1[:, 0:16:2, :],
                            in1=t1[:, 1:16:2, :], op=Alu.add)
    t3 = pool.tile([TR, 4, WC], BF16, tag="t3")
    nc.vector.tensor_tensor(out=t3[:], in0=t2[:, 0:8:2, :],
                            in1=t2[:, 1:8:2, :], op=Alu.add)
    # t3 planes: (vlo_d, vhi_d, vlo_b, vhi_b)
    w2 = pool.tile([TR, 2, WC], F32, tag="w2")
    nc.vector.scalar_tensor_tensor(
        out=w2[:], in0=t3[:, 1::2, :], scalar=256.0,
        in1=t3[:, 0::2, :], op0=Alu.mult, op1=Alu.add)
    wi2 = pool.tile([TR, 2, WC], I32, tag="wi2")
    nc.vector.scalar_tensor_tensor(
        out=wi2[:], in0=t3[:, 0::2, :], scalar=65536.0,
        in1=w2[:], op0=Alu.mult, op1=Alu.add)

    pass2 = _detect_quirky_stacked(nc, pool, wi2)

    mo = pool2.tile([TR, WC], I32, tag="mo")
    nc.vector.tensor_tensor(out=mo[:], in0=pass2[:, 0, :], in1=pass2[:, 1, :],
                            op=Alu.bitwise_or)
    det = pool2.tile([TR, WC], F32, tag="det")
    nc.vector.tensor_scalar(out=det[:], in0=mo[:], scalar1=0, scalar2=None,
                            op0=Alu.not_equal)
    nc.sync.dma_start(out=out_ap[r0:r0 + TR, x0:x0 + WC], in_=det[:])


def build_program(tc, out_ap, img_ap):
    nc = tc.nc
    with tc.tile_pool(name="sh", bufs=2) as shp, \
         tc.tile_pool(name="work", bufs=1) as pool, \
         tc.tile_pool(name="work2", bufs=2) as pool2:
        stride = W + 2 * PAD
        W2 = WC + 2 * PAD
        for t in range(NT):
            r0 = TR * t
            for c in range(NXC):
                x0 = WC * c
                # One folded DMA: partition p gets rows r0+p .. r0+p+6
                # (overlapping-read AP; slot s holds vertical shift dy=s-3).
                sh7 = shp.tile([TR, 7, W2], F32, tag="sh7")
                src = bass.AP(tensor=img_ap.tensor,
                              offset=r0 * stride + x0,
                              ap=[[stride, TR], [stride, 7], [1, W2]])
                nc.sync.dma_start(out=sh7[:], in_=src)
                sh = {dy: sh7[:, dy + PAD, :] for dy in range(-PAD, PAD + 1)}
                _emit_chunk(nc, pool, pool2, sh, out_ap, r0, x0)


_CACHE = {}


def _get_nc():
    if "nc" not in _CACHE:
        nc = bacc.Bacc("TRN2", debug=False, num_devices=NCORES)
        img = nc.dram_tensor("img", [ROWS_PER_CORE + 2 * PAD, W + 2 * PAD], F32,
                             kind="ExternalInput")
        out = nc.dram_tensor("out", [ROWS_PER_CORE, W], F32,
                             kind="ExternalOutput")
        with tile.TileContext(nc) as tc:
            build_program(tc, out.ap(), img.ap())
        nc.compile()
        _CACHE["nc"] = nc
    return _CACHE["nc"]


def shard_inputs(image):
    """image: (4,1,1080,1920) -> list of 8 padded [546, 1926] f32 blocks."""
    blocks = []
    for c in range(NCORES):
        n, half = divmod(c, NCORES // N)
        padded = np.pad(np.asarray(image[n, 0], dtype=np.float32),
                        PAD, mode="edge")
        blocks.append(np.ascontiguousarray(
            padded[half * ROWS_PER_CORE:half * ROWS_PER_CORE + ROWS_PER_CORE + 2 * PAD, :]))
    return blocks


def gather_outputs(results):
    out = np.empty((N, 1, H, W), dtype=np.float32)
    for c in range(NCORES):
        n, half = divmod(c, NCORES // N)
        out[n, 0, half * ROWS_PER_CORE:(half + 1) * ROWS_PER_CORE, :] = \
            results[c]["out"]
    return out


def kernel(image):
    image = np.asarray(image)
    assert image.shape == (N, 1, H, W), image.shape
    nc = _get_nc()
    in_maps = [{"img": b} for b in shard_inputs(image)]
    res = run_bass_kernel_spmd(nc, in_maps, list(range(NCORES)))
    return gather_outputs(res.results)


if __name__ == "__main__":
    img = np.random.default_rng(0).uniform(0, 255, (N, 1, H, W)).astype(np.float32)
    out = kernel(image=img)
    print("kernel ran, out mean:", out.mean())


# revision 19
# speedup vs baseline: 1.0251x; 1.0251x over previous
"""FAST-9 corner score (nn_FASTScore) on 8 Trainium2 NeuronCores.

Matches the jax reference bit-exactly, including the neuron-lowered int32
division rounding quirk in _detect_9_consecutive:
  (x // 2^k) gains +1 iff x >= 2^23 and (k == 0 and x odd, or k >= 2 and
  x mod 2^k == 2^k - 1); k == 1 is exact.
Folding that into the 9-consecutive-bit test over the 24-bit circular
buffer W gives, per window k (window 0 can never pass):
  k == 1 : pass iff bits 1..9 all set            (exact path)
  k >= 2 : quirk_k = b23 & (low k bits of W set)
           pass iff (win==511 & !quirk_k) | (win==510 & quirk_k)
computed bitwise as:
  E9  = 9-run mask;  QMraw = (W+1)^W = 2*lowest_zero_bit - 1
  QM  = QMraw & (all-ones if b23)          # bits k with quirk_k
  ep  = E9 & (((QM|1) & ~2) ^ 0xFFFFFF)    # bit0 killed, bit1 exempt
  qp  = ((r8>>1) & ~W) & (QM & 0xFFFC)     # win==510 & quirk, k in 2..15
  detected iff (ep | qp) != 0
Verified against the reference on all 8.3M pixels (0 mismatches) and the
division rule on 683k boundary samples x 16 divisors.

Device schedule per core (540 output rows of one image):
  host-padded [546, 1926] f32 block; 5 row tiles x 4 x-chunks of [108, 480].
  One folded DMA per chunk loads all 7 vertical shifts (overlapping-read AP).
  GPSIMD: 16 f32 diff planes (single rounding, matches oracle) + 9 of 16
  paired compares. DVE: 7 paired compares, bf16 add-tree pack (weights
  pre-applied in the 2-scalar compare), both polarities stacked [TR,2,WC]
  through a fused scalar_tensor_tensor bitwise detect chain.
"""

import sys

for _p in ("/opt/trn_rl_repo", "/root/.axon_site/_ro/trn_rl_repo"):
    if _p not in sys.path:
        sys.path.append(_p)

import numpy as np

import concourse.bass as bass
import concourse.bacc as bacc
import concourse.tile as tile
from concourse import mybir
from concourse.alu_op_type import AluOpType as Alu
from concourse.bass_utils import run_bass_kernel_spmd

THRESHOLD = 20.0
CIRCLE_OFFSETS = [(0, -3), (1, -3), (2, -2), (3, -1), (3, 0), (3, 1), (2, 2), (1, 3),
                  (0, 3), (-1, 3), (-2, 2), (-3, 1), (-3, 0), (-3, -1), (-2, -2), (-1, -3)]

N, H, W = 4, 1080, 1920
NCORES = 8
ROWS_PER_CORE = (N * H) // NCORES          # 540
PAD = 3
TR = 108                                   # output rows per row-tile
NT = ROWS_PER_CORE // TR                   # 5 row tiles
WC = 480                                   # output cols per x-chunk
NXC = W // WC                              # 4 x chunks

F32 = mybir.dt.float32
BF16 = mybir.dt.bfloat16
I32 = mybir.dt.int32

M24 = 0xFFFFFF


def _detect_quirky_stacked(nc, pool, wi2):
    """Fused quirky detect on stacked words wi2 int32 [TR, 2, WC]."""
    def it(t):
        return pool.tile([TR, 2, WC], I32, tag=t, name=t, bufs=1)

    v = nc.vector

    def stt(out, in0, s, op0, in1, op1):
        # like v.scalar_tensor_tensor but with an int32 immediate (the walrus
        # verifier requires bitvec stt imms to be integer-typed and match
        # src/dst dtype)
        v.add_instruction(mybir.InstTensorScalarPtr(
            name=nc.get_next_instruction_name(),
            is_scalar_tensor_tensor=True,
            op0=op0, op1=op1,
            ins=[v.lower_ap(in0[:]),
                 mybir.ImmediateValue(dtype=mybir.dt.int32, value=s),
                 v.lower_ap(in1[:])],
            outs=[v.lower_ap(out[:])]))

    i0, i1, i2, i3 = (it(f"dq{k}") for k in range(4))

    # pass_k = r8_{k+1} & (W_k ^ quirk_k): a quirky +1 turns win 511 -> 0
    # (no pass) and win 510 -> 511 (pass), i.e. the W_k requirement flips.
    stt(i0, wi2, 1, Alu.logical_shift_right, wi2, Alu.bitwise_and)   # a: runs>=2
    stt(i1, i0, 2, Alu.logical_shift_right, i0, Alu.bitwise_and)     # b4
    stt(i0, i1, 4, Alu.logical_shift_right, i1, Alu.bitwise_and)     # r8
    v.tensor_scalar(out=i2[:], in0=wi2[:], scalar1=1, scalar2=None,
                    op0=Alu.add)                                     # W+1
    v.tensor_tensor(out=i1[:], in0=i2[:], in1=wi2[:],
                    op=Alu.bitwise_xor)                              # QMraw=(W+1)^W
    v.tensor_scalar(out=i2[:], in0=wi2[:], scalar1=8, scalar2=31,
                    op0=Alu.logical_shift_left, op1=Alu.arith_shift_right)  # neg7
    stt(i3, i1, 0xFFFC, Alu.bitwise_and, i2, Alu.bitwise_and)        # QMm (k>=2)
    v.tensor_tensor(out=i2[:], in0=wi2[:], in1=i3[:],
                    op=Alu.bitwise_xor)                              # x = W ^ QMm
    stt(i1, i0, 1, Alu.logical_shift_right, i2, Alu.bitwise_and)     # pass
    return i1


def _emit_chunk(nc, pool, pool2, sh, out_ap, r0, x0):
    """Emit ops for one [TR, WC] output chunk. sh[dy] = shifted img tile APs."""
    center = sh[0][:, PAD:PAD + WC]

    # 16 diff planes, single f32 rounding (must match oracle's circle-center).
    # GPSIMD carries these (engine otherwise idle).
    d = pool.tile([TR, 16, WC], F32, tag="d")
    for j, (dy, dx) in enumerate(CIRCLE_OFFSETS):
        nb = sh[dy][:, PAD + dx:PAD + dx + WC]
        nc.gpsimd.tensor_tensor(out=d[:, j, :], in0=nb, in1=center,
                                op=Alu.subtract)

    # Weighted bit planes, paired (j, j+8) share weight 2^j -> one 2-plane op.
    # Stacked layout: planes 0..15 dark, 16..31 bright.
    bits = pool.tile([TR, 32, WC], BF16, tag="bits")
    for j in range(8):
        wj = float(1 << j)
        nc.gpsimd.tensor_scalar(out=bits[:, j:16:8, :], in0=d[:, j::8, :],
                                scalar1=THRESHOLD, scalar2=wj,
                                op0=Alu.is_ge, op1=Alu.mult)
        nc.gpsimd.tensor_scalar(out=bits[:, 16 + j:32:8, :], in0=d[:, j::8, :],
                                scalar1=-THRESHOLD, scalar2=wj,
                                op0=Alu.is_le, op1=Alu.mult)

    # bf16 add tree (weights pre-applied; all partial sums <= 255 exact)
    t1 = pool.tile([TR, 16, WC], BF16, tag="t1")
    nc.vector.tensor_tensor(out=t1[:], in0=bits[:, 0:32:2, :],
                            in1=bits[:, 1:32:2, :], op=Alu.add)
    t2 = pool.tile([TR, 8, WC], BF16, tag="t2")
    nc.vector.tensor_tensor(out=t2[:], in0=t1[:, 0:16:2, :],
                            in1=t1[:, 1:16:2, :], op=Alu.add)
    t3 = pool.tile([TR, 4, WC], BF16, tag="t3")
    nc.vector.tensor_tensor(out=t3[:], in0=t2[:, 0:8:2, :],
                            in1=t2[:, 1:8:2, :], op=Alu.add)
    # t3 planes: (vlo_d, vhi_d, vlo_b, vhi_b)
    w2 = pool.tile([TR, 2, WC], F32, tag="w2")
    nc.vector.scalar_tensor_tensor(
        out=w2[:], in0=t3[:, 1::2, :], scalar=256.0,
        in1=t3[:, 0::2, :], op0=Alu.mult, op1=Alu.add)
    wi2 = pool.tile([TR, 2, WC], I32, tag="wi2")
    nc.vector.scalar_tensor_tensor(
        out=wi2[:], in0=t3[:, 0::2, :], scalar=65536.0,
        in1=w2[:], op0=Alu.mult, op1=Alu.add)

    pass2 = _detect_quirky_stacked(nc, pool, wi2)

    mo = pool2.tile([TR, WC], I32, tag="mo")
    nc.vector.tensor_tensor(out=mo[:], in0=pass2[:, 0, :], in1=pass2[:, 1, :],
                            op=Alu.bitwise_or)
    nc.vector.tensor_scalar(out=mo[:], in0=mo[:], scalar1=0xFFFE,
                            scalar2=None, op0=Alu.bitwise_and)
    det = pool2.tile([TR, WC], F32, tag="det")
    nc.vector.tensor_scalar(out=det[:], in0=mo[:], scalar1=0, scalar2=None,
                            op0=Alu.not_equal)
    nc.sync.dma_start(out=out_ap[r0:r0 + TR, x0:x0 + WC], in_=det[:])


def build_program(tc, out_ap, img_ap):
    nc = tc.nc
    with tc.tile_pool(name="sh", bufs=2) as shp, \
         tc.tile_pool(name="work", bufs=1) as pool, \
         tc.tile_pool(name="work2", bufs=2) as pool2:
        stride = W + 2 * PAD
        W2 = WC + 2 * PAD
        for t in range(NT):
            r0 = TR * t
            for c in range(NXC):
                x0 = WC * c
                # One folded DMA: partition p gets rows r0+p .. r0+p+6
                # (overlapping-read AP; slot s holds vertical shift dy=s-3).
                sh7 = shp.tile([TR, 7, W2], F32, tag="sh7")
                src = bass.AP(tensor=img_ap.tensor,
                              offset=r0 * stride + x0,
                              ap=[[stride, TR], [stride, 7], [1, W2]])
                nc.sync.dma_start(out=sh7[:], in_=src)
                sh = {dy: sh7[:, dy + PAD, :] for dy in range(-PAD, PAD + 1)}
                _emit_chunk(nc, pool, pool2, sh, out_ap, r0, x0)


_CACHE = {}


def _get_nc():
    if "nc" not in _CACHE:
        nc = bacc.Bacc("TRN2", debug=False, num_devices=NCORES)
        img = nc.dram_tensor("img", [ROWS_PER_CORE + 2 * PAD, W + 2 * PAD], F32,
                             kind="ExternalInput")
        out = nc.dram_tensor("out", [ROWS_PER_CORE, W], F32,
                             kind="ExternalOutput")
        with tile.TileContext(nc) as tc:
            build_program(tc, out.ap(), img.ap())
        nc.compile()
        _CACHE["nc"] = nc
    return _CACHE["nc"]


def shard_inputs(image):
    """image: (4,1,1080,1920) -> list of 8 padded [546, 1926] f32 blocks."""
    blocks = []
    for c in range(NCORES):
        n, half = divmod(c, NCORES // N)
        padded = np.pad(np.asarray(image[n, 0], dtype=np.float32),
                        PAD, mode="edge")
        blocks.append(np.ascontiguousarray(
            padded[half * ROWS_PER_CORE:half * ROWS_PER_CORE + ROWS_PER_CORE + 2 * PAD, :]))
    return blocks


def gather_outputs(results):
    out = np.empty((N, 1, H, W), dtype=np.float32)
    for c in range(NCORES):
        n, half = divmod(c, NCORES // N)
        out[n, 0, half * ROWS_PER_CORE:(half + 1) * ROWS_PER_CORE, :] = \
            results[c]["out"]
    return out


def kernel(image):
    image = np.asarray(image)
    assert image.shape == (N, 1, H, W), image.shape
    nc = _get_nc()
    in_maps = [{"img": b} for b in shard_inputs(image)]
    res = run_bass_kernel_spmd(nc, in_maps, list(range(NCORES)))
    return gather_outputs(res.results)


if __name__ == "__main__":
    img = np.random.default_rng(0).uniform(0, 255, (N, 1, H, W)).astype(np.float32)
    out = kernel(image=img)
    print("kernel ran, out mean:", out.mean())
